# revision 14
# baseline (speedup 1.0000x reference)
"""PASA group-softmax high-pass downsample kernel for 8 Trainium2 NeuronCores.

Reference computation (n=4, c=64, h=w=128, G=2 groups, K=3, stride 2):
  xp     = reflect_pad(x, 1)
  sigma  = conv3x3(xp, conv_w)                    # [n, 18, h, w]
  sigma  = sigma * bn_scale + bn_shift            # BN (inference)
  sigma  = softmax(sigma, axis=1)                 # over all 18 channels
  sigma  = onehot(center) - sigma                 # high-pass
  out[n,g,c,i,j] = sum_k patches[n,g,c,k,i,j] * sigma[n,g,k,i,j]
  return out[:, :, ::2, ::2]                      # [4, 64, 64, 64]

Device mapping (per core = one (image, 32-row half) shard):
  x is host-prepped into 3 "dx-planes" [128, 33, 3, 64] bf16: partition
  p = 64*s + c holds channel c of sub-half s; plane row r, plane dx, col j
  = xp[c, 32s + r, 2j + dx].  Every conv tap and every apply tap is then a
  unit-stride slice (DVE 2x mode capable).
  conv    -> one PSUM tile sigma [128, 512]: col-group q = s + 2*ch holds
             the 18 sigma channels for (sub-half s, 8-row chunk ch).
  exp     -> e = exp(sigma + bn_shift) bf16 (ACT, bias per partition).
  D       -> sel-matmul -> d [4, 512]; r = approx recip (DVE, 1 pass);
             cast bf16; rbig = rsel-matmul broadcast; f = e * rbig (bf16).
  apply   -> per tap k: 2 CONCURRENT row-tiled matmuls (contract 64) expand
             f -> ebig [128, 1024] PSUM; ACT copies to SBUF bf16 (7 taps;
             2 taps multiply straight from PSUM); DVE mul prod = patch * f;
             PE accumulates acc += (-I) @ prod.  acc starts from (+I) @ xc
             so acc ends as y = xc - sum_k patch_k * f_k directly.
  out     -> DVE copy acc -> bf16, 2 chunked stores; host casts to f32.
"""

import os
import ml_dtypes
import numpy as np

import concourse.bass as bass
import concourse.tile as tile
from concourse import bacc, mybir
from concourse.bass_utils import run_bass_kernel_spmd

F32 = mybir.dt.float32
BF16 = mybir.dt.bfloat16

N, C, H, W = 4, 64, 128, 128
G, K = 2, 3
K2 = K * K
EPS = 1e-5
NCORES = 8
HO, WO = H // 2, W // 2            # 64 x 64 output spatial
ROWS_PER_CORE = HO // 2            # 32 output rows per core
ROWS_SUB = ROWS_PER_CORE // 2      # 16 output rows per sub-half (s=0,1)
PL_R = 2 * ROWS_SUB + 1            # 33 plane rows per sub-half
POS_SUB = ROWS_SUB * WO            # 1024 positions per sub-half
CHUNK_ROWS = ROWS_SUB // 2         # 8 output rows per psum chunk
CHUNK = CHUNK_ROWS * WO            # 512 positions per chunk

# wpack tensor column layout (bf16): idents + conv weights + sel
OFF_NEGI = 0
OFF_POSI = 128
OFF_WTS = 256                      # [9, 32] flattened
OFF_SEL = OFF_WTS + K2 * 32        # 544
WPW = OFF_SEL + 4                  # 548
# epack tensor column layout (bf16): esel + rsel
OFF_ESEL = 0                       # [9, 128] flattened
OFF_RSEL = K2 * 128                # 1152 (rows 0..3 only)
EPW = OFF_RSEL + 128               # 1280

NJUNK = 18                         # PE warm-up matmuls (N=128 each)
HEAD_JUNK = (6, 10, 6)             # junk MMs after xc / after d / after rbig
DIRECT_TAPS = (0, 8)               # taps whose mul reads ebig from PSUM

_compiled = None


def _build_program():
    nc = bacc.Bacc(
        "TRN2", target_bir_lowering=False, debug=False, num_devices=NCORES
    )

    xpl = nc.dram_tensor("xpl", [128, PL_R, K, WO], BF16, kind="ExternalInput")
    wpack = nc.dram_tensor("wpack", [128, WPW], BF16, kind="ExternalInput")
    epack = nc.dram_tensor("epack", [128, EPW], BF16, kind="ExternalInput")
    bias = nc.dram_tensor("bias", [128, 1], F32, kind="ExternalInput")
    y = nc.dram_tensor("y", [128, POS_SUB], BF16, kind="ExternalOutput")
    warm_out = nc.dram_tensor("warm_out", [1, 2], F32, kind="ExternalOutput")

    with tile.TileContext(nc) as tc:
        with (
            tc.tile_pool(name="singles", bufs=1) as singles,
            tc.tile_pool(name="psum", bufs=1, space="PSUM") as psum,
            tc.tile_pool(name="ebig", bufs=2, space="PSUM") as ebig_pool,
            tc.tile_pool(name="esb", bufs=2) as esb_pool,
            tc.tile_pool(name="prod", bufs=3) as prod_pool,
            tc.tile_pool(name="work", bufs=2) as work,
        ):
            # ---- input DMAs -------------------------------------------------
            # sync ring: wpack (idents+wts, needed first), bias, then the two
            # sub-half-B plane chunks; scalar ring: sub-half-A chunks, then
            # epack (esel, only needed at the apply phase).
            w_pk = singles.tile([128, WPW], BF16)
            nc.sync.dma_start(w_pk[:], wpack.ap())
            bias_sb = singles.tile([128, 1], F32)
            nc.sync.dma_start(bias_sb[:], bias.ap())

            x_sb = singles.tile([128, PL_R, K, WO], BF16)
            for ch in range(2):
                r0 = 0 if ch == 0 else 2 * CHUNK_ROWS + 1
                r1 = 2 * CHUNK_ROWS + 1 if ch == 0 else PL_R
                nc.scalar.dma_start(x_sb[0:64, r0:r1], xpl.ap()[0:64, r0:r1])
                nc.sync.dma_start(x_sb[64:128, r0:r1], xpl.ap()[64:128, r0:r1])
            e_pk = singles.tile([128, EPW], BF16)
            nc.scalar.dma_start(e_pk[:], epack.ap())

            # prewarm ACT's exp table off the critical path
            warm_in = work.tile([1, 1], F32, tag="warm_in")
            nc.vector.memset(warm_in[:], 0.25)
            warm_e = work.tile([1, 1], F32, tag="warm_e")
            nc.scalar.activation(warm_e[:], warm_in[:],
                                 mybir.ActivationFunctionType.Exp)

            negI = w_pk[:, OFF_NEGI : OFF_NEGI + 128]
            posI = w_pk[:, OFF_POSI : OFF_POSI + 128]
            w_sb = w_pk[:, OFF_WTS : OFF_WTS + K2 * 32].rearrange(
                "p (k o) -> p k o", k=K2
            )
            sel_sb = w_pk[:, OFF_SEL : OFF_SEL + 4]
            esel_sb = e_pk[:, OFF_ESEL : OFF_ESEL + K2 * 128].rearrange(
                "p (k o) -> p k o", k=K2
            )
            rsel_sb = e_pk[0:4, OFF_RSEL : OFF_RSEL + 128]
            # flattened 3D view of the planes: row index = 3*r + dx
            xv = x_sb[:].rearrange("p r d c -> p (r d) c")

            # ---- PE warm-up: junk matmuls on the consts keep HAM busy while
            # the x planes stream in.  Writes the (later reused) acc banks.
            acc_ps = psum.tile([128, POS_SUB], F32, tag="acc")
            for i in range(NJUNK):
                nc.tensor.matmul(acc_ps[:, 0:128], posI, posI,
                                 start=(i == 0), stop=(i == NJUNK - 1),
                                 skip_group_check=True)
            warm_sb = work.tile([1, 2], F32, tag="warm_sb")
            nc.vector.tensor_copy(warm_sb[:], acc_ps[0:1, 0:2])
            nc.sync.dma_start(warm_out.ap(), warm_sb[:])

            # ---- conv: 9 taps x 4 col-groups (4-way tile packing) ----------
            sigma_ps = psum.tile([128, CHUNK], F32, tag="sigma")
            for k in range(K2):
                dy, dx = k // K, k % K
                for ch in range(2):
                    for s in range(2):
                        p0 = 64 * s
                        q = s + 2 * ch
                        a = 3 * (dy + 2 * CHUNK_ROWS * ch) + dx
                        nc.tensor.matmul(
                            sigma_ps[32 * q : 32 * q + 32, :],
                            w_sb[p0 : p0 + 64, k, :],
                            xv[p0 : p0 + 64, a : a + 6 * CHUNK_ROWS - 5 : 6, :],
                            start=(k == 0),
                            stop=(k == K2 - 1),
                            tile_position=(p0, 32 * q),
                            skip_group_check=True,
                        )

            # ---- start the y accumulation with +I @ xc (center patch) ------
            # (placed right after conv so PE stays busy during the head)
            for ch in range(2):
                a = 3 * (1 + 2 * CHUNK_ROWS * ch) + 1
                nc.tensor.matmul(
                    acc_ps[:, CHUNK * ch : CHUNK * (ch + 1)],
                    posI,
                    xv[:, a : a + 6 * CHUNK_ROWS - 5 : 6, :],
                    start=True,
                    stop=False,
                    skip_group_check=True,
                )

            # junk tile in the ebig pool: HAM-warmth filler during the head
            junk_ps = ebig_pool.tile([128, POS_SUB], F32, name="junkhead",
                                     tag="ebig")

            def head_junk(n):
                for i in range(n):
                    nc.tensor.matmul(junk_ps[:, 0:128], posI, posI,
                                     start=True, stop=True,
                                     skip_group_check=True)

            head_junk(HEAD_JUNK[0])

            # ---- softmax head ----------------------------------------------
            e_sb = singles.tile([128, CHUNK], BF16)
            nc.scalar.activation(
                e_sb[:], sigma_ps[:], mybir.ActivationFunctionType.Exp,
                bias=bias_sb[:], scale=1.0,
            )
            d_ps = psum.tile([4, CHUNK], F32, tag="dr")
            nc.tensor.matmul(d_ps[:], sel_sb, e_sb[:])
            head_junk(HEAD_JUNK[1])
            r_sb = singles.tile([4, CHUNK], F32)
            nc.vector.reciprocal_approx_fast(r_sb[:], d_ps[:])
            r_bf = singles.tile([4, CHUNK], BF16)
            nc.scalar.copy(r_bf[:], r_sb[:])
            rbig_ps = psum.tile([128, CHUNK], F32, tag="dr")
            nc.tensor.matmul(rbig_ps[:], rsel_sb, r_bf[:])
            head_junk(HEAD_JUNK[2])
            f_sb = singles.tile([128, CHUNK], BF16)
            nc.vector.tensor_mul(f_sb[:], e_sb[:], rbig_ps[:])

            # ---- apply: per tap broadcast + multiply + PE accumulate -------
            prods = []
            for k in range(K2):
                dy, dx = k // K, k % K
                ebig = ebig_pool.tile([128, POS_SUB], F32,
                                      name=f"ebig{k}", tag="ebig")
                for ch in range(2):
                    nc.tensor.matmul(
                        ebig[:, CHUNK * ch : CHUNK * (ch + 1)],
                        esel_sb[64 * ch : 64 * ch + 64, k, :],
                        f_sb[64 * ch : 64 * ch + 64, :],
                        tile_position=(64 * ch, 0),
                        skip_group_check=True,
                    )
                b = 3 * dy + dx
                patch = xv[:, b : b + 6 * ROWS_SUB - 5 : 6, :]
                prod = prod_pool.tile([128, ROWS_SUB, WO], BF16,
                                      name=f"prod{k}", tag="prod")
                ebig3 = ebig[:].rearrange("p (r c) -> p r c", r=ROWS_SUB)
                if k in DIRECT_TAPS:
                    nc.vector.tensor_mul(prod[:], patch, ebig3)
                else:
                    ebig_sb = esb_pool.tile([128, ROWS_SUB, WO], BF16,
                                            name=f"esb{k}", tag="esb")
                    nc.scalar.copy(ebig_sb[:], ebig3)
                    nc.vector.tensor_mul(prod[:], patch, ebig_sb[:])
                prods.append(prod)
                if k >= 1:
                    pprev = prods[k - 1][:].rearrange("p r c -> p (r c)")
                    for ch in range(2):
                        nc.tensor.matmul(
                            acc_ps[:, CHUNK * ch : CHUNK * (ch + 1)],
                            negI,
                            pprev[:, CHUNK * ch : CHUNK * (ch + 1)],
                            start=False,
                            stop=False,
                            skip_group_check=True,
                        )
            plast = prods[K2 - 1][:].rearrange("p r c -> p (r c)")
            for ch in range(2):
                nc.tensor.matmul(
                    acc_ps[:, CHUNK * ch : CHUNK * (ch + 1)],
                    negI,
                    plast[:, CHUNK * ch : CHUNK * (ch + 1)],
                    start=False,
                    stop=(ch == 1),
                    skip_group_check=True,
                )

            # ---- acc_ps now holds y; cast + store (chunk-pipelined) --------
            y_sb = work.tile([128, POS_SUB], BF16, tag="ysb")
            for ch in range(2):
                cs = slice(CHUNK * ch, CHUNK * (ch + 1))
                nc.vector.tensor_copy(y_sb[:, cs], acc_ps[:, cs])
                eng = nc.sync if ch == 0 else nc.scalar
                eng.dma_start(y.ap()[:, cs], y_sb[:, cs])

    nc.compile()
    return nc


def _host_inputs(x, conv_w, gamma, beta, running_mean, running_var):
    """Per-core input dicts: BN folding + reflect pad + dx-plane layout."""
    scale = gamma / np.sqrt(running_var + EPS)
    shift = beta - running_mean * scale

    # conv weights as lhsT [tap, c, o] * bn_scale, padded to 32 outs, dup'd
    w_scaled = conv_w * scale[:, None, None, None]           # [18, 64, 3, 3]
    wl = np.transpose(w_scaled, (2, 3, 1, 0)).reshape(K2, C, G * K2)
    wl32 = np.zeros((K2, C, 32), np.float32)
    wl32[:, :, : G * K2] = wl
    wts = np.ascontiguousarray(
        np.concatenate([wl32, wl32], axis=1).transpose(1, 0, 2)
    ).reshape(128, K2 * 32)

    wpk = np.zeros((128, WPW), np.float32)
    wpk[:, OFF_NEGI : OFF_NEGI + 128] = -np.eye(128)
    wpk[:, OFF_POSI : OFF_POSI + 128] = np.eye(128)
    wpk[:, OFF_WTS : OFF_WTS + K2 * 32] = wts
    epk = np.zeros((128, EPW), np.float32)
    for q in range(4):
        wpk[32 * q : 32 * q + G * K2, OFF_SEL + q] = 1.0           # sel
        epk[q, OFF_RSEL + 32 * q : OFF_RSEL + 32 * q + G * K2] = 1.0
    esel = np.zeros((128, K2, 128), np.float32)
    for k in range(K2):
        for p in range(128):
            s, g = p // 64, (p % 64) // 32
            for ch in range(2):
                esel[32 * (s + 2 * ch) + K2 * g + k, k, p] = 1.0
    epk[:, OFF_ESEL : OFF_ESEL + K2 * 128] = esel.reshape(128, K2 * 128)
    wpk = wpk.astype(ml_dtypes.bfloat16)
    epk = epk.astype(ml_dtypes.bfloat16)

    bias = np.zeros((128, 1), np.float32)
    for q in range(4):
        bias[32 * q : 32 * q + G * K2, 0] = shift

    xpad = np.pad(x, ((0, 0), (0, 0), (1, 1), (1, 1)), mode="reflect")

    in_maps = []
    for core in range(NCORES):
        n, h = core // 2, core % 2
        r0 = 64 * h
        xpl = np.zeros((128, PL_R, K, WO), np.float32)
        for s in range(2):
            sl = xpad[n, :, r0 + 32 * s : r0 + 32 * s + PL_R, :]  # [64,33,130]
            for dx in range(K):
                xpl[64 * s : 64 * s + 64, :, dx, :] = sl[:, :, dx::2][:, :, :WO]
        in_maps.append(
            {"xpl": xpl.astype(ml_dtypes.bfloat16), "wpack": wpk,
             "epack": epk, "bias": bias}
        )
    return in_maps


def _gather_output(results):
    out = np.empty((N, C, HO, WO), np.float32)
    for core, res in enumerate(results):
        n, h = core // 2, core % 2
        ycore = np.asarray(res["y"]).astype(np.float32).reshape(
            2, C, ROWS_SUB, WO
        )
        out[n, :, 32 * h : 32 * h + ROWS_SUB, :] = ycore[0]
        out[n, :, 32 * h + ROWS_SUB : 32 * h + 2 * ROWS_SUB, :] = ycore[1]
    return out


def _ensure_ntff_hook():
    """Install the axon NTFF profile hook if the image's antenv lacks it."""
    try:
        from antenv import axon_hooks  # noqa: F401
        return
    except ImportError:
        pass
    try:
        import sys
        import types

        import antenv
        from trn_agent_boot.trn_boot import _ntff_profile_via_ctypes

        hook = _ntff_profile_via_ctypes("/opt/axon/libaxon_pjrt.so")
        mod = types.ModuleType("antenv.axon_hooks")
        state = {"hook": hook}
        mod.get_axon_ntff_profile_hook = lambda: state["hook"]
        mod.set_axon_ntff_profile_hook = lambda h: state.update(hook=h)
        sys.modules["antenv.axon_hooks"] = mod
        antenv.axon_hooks = mod
    except Exception:
        pass


def kernel(x, conv_w, gamma, beta, running_mean, running_var):
    global _compiled
    x = np.asarray(x, np.float32)
    conv_w = np.asarray(conv_w, np.float32)
    gamma = np.asarray(gamma, np.float32)
    beta = np.asarray(beta, np.float32)
    running_mean = np.asarray(running_mean, np.float32)
    running_var = np.asarray(running_var, np.float32)

    if _compiled is None:
        _compiled = _build_program()
    nc = _compiled

    in_maps = _host_inputs(x, conv_w, gamma, beta, running_mean, running_var)
    trace = bool(int(os.environ.get("PASA_TRACE", "0")))
    if trace:
        _ensure_ntff_hook()
    res = run_bass_kernel_spmd(
        nc, in_maps, core_ids=list(range(NCORES)), trace=trace
    )
    kernel.last_results = res
    return _gather_output(res.results)


if __name__ == "__main__":
    # quick CoreSim check of core 0 against a numpy re-implementation
    from concourse.bass_interp import CoreSim

    rng = np.random.default_rng(0)
    x = rng.standard_normal((N, C, H, W)).astype(np.float32)
    conv_w = (rng.standard_normal((G * K2, C, K, K)).astype(np.float32)
              * np.sqrt(2.0 / (G * K2 * K * K)))
    gamma = rng.uniform(0.5, 1.5, G * K2).astype(np.float32)
    beta = (rng.standard_normal(G * K2) * 0.1).astype(np.float32)
    rmean = (rng.standard_normal(G * K2) * 0.1).astype(np.float32)
    rvar = rng.uniform(0.5, 1.5, G * K2).astype(np.float32)

    nc = _build_program()
    in_maps = _host_inputs(x, conv_w, gamma, beta, rmean, rvar)
    sim = CoreSim(nc)
    for kk, v in in_maps[0].items():
        sim.tensor(kk)[:] = v
    sim.simulate(check_with_hw=False)
    ysim = np.asarray(sim.tensor("y")).astype(np.float32).reshape(
        2, C, ROWS_SUB, WO
    )

    # numpy reference for core 0 region (image 0, output rows 0..32)
    scale = gamma / np.sqrt(rvar + EPS)
    shift = beta - rmean * scale
    xpad = np.pad(x[0], ((0, 0), (1, 1), (1, 1)), mode="reflect")
    sig = np.zeros((G * K2, 32, WO), np.float32)
    for o in range(G * K2):
        for dy in range(K):
            for dx in range(K):
                sig[o] += np.einsum(
                    "crw->rw",
                    conv_w[o, :, dy, dx][:, None, None]
                    * xpad[:, dy : dy + 64 : 2, dx : dx + 128 : 2],
                )
    sig = sig * scale[:, None, None] + shift[:, None, None]
    e = np.exp(sig)
    r = 1.0 / e.sum(0)
    acc = np.zeros((C, 32, WO), np.float32)
    for g in range(G):
        for k in range(K2):
            dy, dx = k // K, k % K
            acc[32 * g : 32 * g + 32] += (
                xpad[32 * g : 32 * g + 32, dy : dy + 64 : 2, dx : dx + 128 : 2]
                * e[g * K2 + k][None]
            )
    ref = (xpad[:, 1:65:2, 1:129:2] - acc * r[None]).astype(np.float32)

    got = np.concatenate([ysim[0], ysim[1]], axis=1)
    err = np.abs(got - ref).max() / np.abs(ref).max()
    print("sim rel err:", err)
